# revision 8
# baseline (speedup 1.0000x reference)
"""Trainium2 Bass kernel for nn_ConvDY2d (dynamic-weight 3x3 conv, CondConv-style).

Reference computation (B=16, C=O=256, H=W=64, K=4 mixing kernels):
  attn  = softmax(MLP(global_avg_pool(x)) / 30)            # [B, 4]
  w_mix = einsum('bk,koihw->boihw', attn, w_dyn)           # per-sample 3x3 conv kernel
  out[b] = conv2d(x[b], w_mix[b], padding=1)

Strategy: data-parallel over batch, 2 samples per NeuronCore across 8 cores.
Per core, the conv is an implicit GEMM: for each (out-channel block, 8-row
group) a [128, 512] PSUM tile accumulates 18 matmuls (2 c-blocks x 9 taps)
whose rhs are contiguous 512-element slices of a row-padded input image
([128c, 4226]).  Column wrap-around at row edges is fixed up afterwards by
subtracting border corrections computed with 12 strided-rhs matmuls per
output block.

Startup-latency-optimized schedule:
  - x[b0] loads first (row chunks); pooling partials chase the DMA chunks
    (DVE cb0 / ACT cb1), dep-chained in order so the tile scheduler cannot
    reorder them behind later DMAs that recycle the same semaphores.
  - wdyn is loaded split by (cblock, dy-row, k): the first mix chunk only
    needs 0.75MB of wdyn, so conv matmuls start ~19us instead of ~32us.
  - weight mixing uses tensor_scalar (4x DVE mode) + tensor_tensor (2x)
    trees in bf16: 7 ops/chunk instead of a 4-long scalar_tensor_tensor
    chain at 1x.
  - the conv runs in PSUM groups of 5 tiles with per-(cb,dy) passes so the
    matmul stream chases the mix chunks without stalling.
"""

import sys

if "/opt/trn_rl_repo" not in sys.path:
    sys.path.insert(0, "/opt/trn_rl_repo")

import numpy as np

B, C, H, W = 16, 256, 64, 64
O, K, KS = 256, 4, 3
MID = C // 4
INV_DELTA = 1.0 / 30.0
NCORES = 8
NB = B // NCORES            # samples per core
NPOS = KS * KS              # 9 taps
FPAD = 1 + 66 * W + 1       # padded image free size: 4226
ROW0 = 65                   # flat offset of input row 0 (= 1 + 1*64)

# x chunk row-splits per c-block: small final chunk so the last pooling
# partial is cheap and attention comes off the critical path.
XCHUNKS = [(0, 20), (20, 20), (40, 20), (60, 4)]

_CACHE = {}


def _build_nc():
    import concourse.bacc as bacc
    import concourse.tile as tile
    from concourse import mybir
    from concourse.tile_rust import add_dep_helper

    f32 = mybir.dt.float32
    bf16 = mybir.dt.bfloat16
    AX = mybir.AxisListType
    ALU = mybir.AluOpType
    ACTF = mybir.ActivationFunctionType

    nc = bacc.Bacc(target_bir_lowering=False, debug=False)

    x_d = nc.dram_tensor("x", [NB, C, H, W], bf16, kind="ExternalInput").ap()
    wd_d = nc.dram_tensor("wdynT", [K, NPOS, C, O], bf16, kind="ExternalInput").ap()
    fc1wT_d = nc.dram_tensor("fc1wT", [C, MID], f32, kind="ExternalInput").ap()
    fc1b_d = nc.dram_tensor("fc1b", [1, MID], f32, kind="ExternalInput").ap()
    fc2aug_d = nc.dram_tensor("fc2aug", [MID + 1, K], f32, kind="ExternalInput").ap()
    out_d = nc.dram_tensor("out", [NB, O, H, W], f32, kind="ExternalOutput").ap()

    with tile.TileContext(nc) as tc:
        with (
            tc.tile_pool(name="consts", bufs=1) as constp,
            tc.tile_pool(name="wdyn", bufs=1) as wdynp,
            tc.tile_pool(name="wmix", bufs=1) as wmixp,
            tc.tile_pool(name="xpad", bufs=1) as xpadp,
            tc.tile_pool(name="osb", bufs=6) as osbp,
            tc.tile_pool(name="convps", bufs=5, space="PSUM") as convps,
            tc.tile_pool(name="corrps", bufs=2, space="PSUM") as corrps,
            tc.tile_pool(name="smallps", bufs=1, space="PSUM") as smallps,
        ):
            # ---------------- tiny consts FIRST (ahead of bulk DMA) ----------
            fc1wT_sb = constp.tile([128, 2 * MID], f32, tag="fc1w", name="fc1wT_sb")
            for cb in range(2):
                nc.sync.dma_start(
                    fc1wT_sb[:, cb * MID : (cb + 1) * MID],
                    fc1wT_d[cb * 128 : (cb + 1) * 128, :],
                )
            fc1b_sb = constp.tile([1, MID], f32, tag="fc1b", name="fc1b_sb")
            nc.sync.dma_start(fc1b_sb, fc1b_d)
            fc2aug_sb = constp.tile([MID + 1, K], f32, tag="fc2", name="fc2aug_sb")
            nc.sync.dma_start(fc2aug_sb, fc2aug_d)

            ones_sb = constp.tile([1, 128], f32, tag="ones", name="ones_sb")
            nc.gpsimd.memset(ones_sb, 1.0)
            act_dummy = constp.tile([128, 20 * W], bf16, tag="actdum", name="act_dummy")

            # xpad tiles + pad memsets for both samples up front (gpsimd idle)
            xpad = [[None, None] for _ in range(NB)]
            for b in range(NB):
                for cb in range(2):
                    t = xpadp.tile([128, FPAD], bf16, tag=f"xpad{b}{cb}", name=f"xpad{b}{cb}")
                    nc.gpsimd.memset(t[:, 0:ROW0], 0.0)
                    nc.gpsimd.memset(t[:, ROW0 + H * W : FPAD], 0.0)
                    xpad[b][cb] = t

            def load_x(b):
                # interleave c-block chunks so both pooling engines get data
                for r0, nr in XCHUNKS:
                    for cb in range(2):
                        nc.sync.dma_start(
                            xpad[b][cb][:, ROW0 + r0 * W : ROW0 + (r0 + nr) * W],
                            x_d[
                                b, cb * 128 : (cb + 1) * 128, r0 : r0 + nr, :
                            ].rearrange("c h w -> c (h w)"),
                        )

            load_x(0)

            # ---------------- pooling + attention (sample 0) -----------------
            # Partials are dep-chained in chunk order: the scheduler would
            # otherwise reorder them and regenerate semaphore targets against
            # later DMAs that recycle the same semaphores (false deps).
            def pool_sample(b, engines):
                pooled = [None, None]
                nch = len(XCHUNKS)
                prev = [None, None]
                for h, (r0, nr) in enumerate(XCHUNKS):
                    for cb in range(2):
                        if h == 0:
                            pp = constp.tile(
                                [128, nch], f32, tag=f"pp{b}{cb}", name=f"pp{b}{cb}"
                            )
                            if cb == 0:
                                pp0 = pp
                            else:
                                pp1 = pp
                        pp = pp0 if cb == 0 else pp1
                        src = xpad[b][cb][:, ROW0 + r0 * W : ROW0 + (r0 + nr) * W]
                        if engines[cb] == "dve":
                            r = nc.vector.reduce_sum(pp[:, h : h + 1], src, AX.X)
                        else:
                            r = nc.scalar.activation(
                                act_dummy[:, 0 : nr * W], src, ACTF.Copy,
                                accum_out=pp[:, h : h + 1],
                            )
                        if prev[cb] is not None:
                            add_dep_helper(r.ins, prev[cb].ins, sync=True,
                                           reason="pool chunk order")
                        prev[cb] = r
                for cb in range(2):
                    pp = pp0 if cb == 0 else pp1
                    p = constp.tile([128, 1], f32, tag=f"pool{b}{cb}", name=f"pooled{b}{cb}")
                    r = nc.vector.reduce_sum(p, pp, AX.X)
                    if engines[cb] == "dve":
                        add_dep_helper(r.ins, prev[cb].ins, sync=True,
                                       reason="pool final order")
                    pooled[cb] = p
                return pooled

            def attn_mlp(b, pooled, first_dep):
                hid_ps = smallps.tile([MID, 1], f32, tag="small", name=f"hid_ps{b}")
                first_mm = None
                for cb in range(2):
                    mm = nc.tensor.matmul(
                        hid_ps,
                        fc1wT_sb[:, cb * MID : (cb + 1) * MID],
                        pooled[cb],
                        start=(cb == 0),
                        stop=False,
                    )
                    if first_mm is None:
                        first_mm = mm
                        if first_dep is not None:
                            add_dep_helper(mm.ins, first_dep.ins, sync=True,
                                           reason="PE order for MLP")
                nc.tensor.matmul(hid_ps, fc1b_sb, ones_sb[:, 0:1], start=False, stop=True)

                hid_sb = constp.tile([MID + 1, 1], f32, tag=f"hid{b}", name=f"hid_sb{b}")
                nc.gpsimd.memset(hid_sb[MID : MID + 1, :], 1.0)
                nc.scalar.activation(hid_sb[0:MID, :], hid_ps, ACTF.Relu)

                lg_ps = smallps.tile([1, K], f32, tag="small", name=f"lg_ps{b}")
                nc.tensor.matmul(lg_ps, hid_sb, fc2aug_sb, start=True, stop=True)

                ex = constp.tile([1, K], f32, tag=f"ex{b}", name=f"ex{b}")
                sm = constp.tile([1, 1], f32, tag=f"sm{b}", name=f"sm{b}")
                nc.scalar.activation(ex, lg_ps, ACTF.Exp, accum_out=sm)
                rc = constp.tile([1, 1], f32, tag=f"rc{b}", name=f"rc{b}")
                nc.vector.reciprocal(rc, sm)
                attn = constp.tile([1, K], f32, tag=f"at{b}", name=f"attn{b}")
                nc.vector.tensor_scalar_mul(attn, ex, rc)
                attn_bc = constp.tile([128, K], f32, tag=f"abc{b}", name=f"attn_bc{b}")
                nc.gpsimd.partition_broadcast(attn_bc, attn)
                return attn_bc

            pooled0 = pool_sample(0, ("dve", "act"))
            attn_bc0 = attn_mlp(0, pooled0, None)

            # ---------------- wdyn loads, split by (cb, dy-row, k) -----------
            # First 4 DMAs (cb0 dy0, all k) are all the first mix chunk needs.
            wdyn = [[None, None] for _ in range(K)]
            for cb in range(2):
                for k in range(K):
                    wdyn[k][cb] = wdynp.tile(
                        [128, NPOS * O], bf16, tag=f"wd{k}{cb}", name=f"wd{k}{cb}"
                    )
            for cb in range(2):
                for dy in range(KS):
                    for k in range(K):
                        nc.sync.dma_start(
                            wdyn[k][cb].rearrange("c (p o) -> c p o", o=O)[
                                :, dy * KS : (dy + 1) * KS, :
                            ],
                            wd_d[
                                k, dy * KS : (dy + 1) * KS, cb * 128 : (cb + 1) * 128, :
                            ].transpose([1, 0, 2]),
                        )

            # ---------------- weight mixing: bf16 ts/tt trees on DVE ---------
            # Per (cb, dy) chunk of [128, 768]:
            #   wm = a0*w0; s1 = a1*w1; wm += s1; s1 = a2*w2; s2 = a3*w3;
            #   s1 += s2; wm += s1          (4x tensor_scalar, 2x tensor_tensor)
            mix_s1 = constp.tile([128, KS * O], bf16, tag="mixs1", name="mix_s1")
            mix_s2 = constp.tile([128, KS * O], bf16, tag="mixs2", name="mix_s2")
            wmix = [[None, None] for _ in range(NB)]
            mix_last = [None]

            def mix_sample(b, attn_bc):
                for cb in range(2):
                    wm = wmixp.tile(
                        [128, NPOS * O], bf16, tag=f"wm{b}{cb}", name=f"wmix{b}{cb}"
                    )
                    for dy in range(KS):
                        lo, hi = dy * KS * O, (dy + 1) * KS * O
                        wmh = wm[:, lo:hi]
                        first = nc.vector.tensor_scalar_mul(
                            wmh, wdyn[0][cb][:, lo:hi], attn_bc[:, 0:1]
                        )
                        if mix_last[0] is not None:
                            add_dep_helper(first.ins, mix_last[0].ins, sync=True,
                                           reason="mix chunk order")
                        nc.vector.tensor_scalar_mul(
                            mix_s1, wdyn[1][cb][:, lo:hi], attn_bc[:, 1:2]
                        )
                        nc.vector.tensor_tensor(wmh, wmh, mix_s1, op=ALU.add)
                        nc.vector.tensor_scalar_mul(
                            mix_s1, wdyn[2][cb][:, lo:hi], attn_bc[:, 2:3]
                        )
                        nc.vector.tensor_scalar_mul(
                            mix_s2, wdyn[3][cb][:, lo:hi], attn_bc[:, 3:4]
                        )
                        nc.vector.tensor_tensor(mix_s1, mix_s1, mix_s2, op=ALU.add)
                        mix_last[0] = nc.vector.tensor_tensor(
                            wmh, wmh, mix_s1, op=ALU.add
                        )
                    wmix[b][cb] = wm

            mix_sample(0, attn_bc0)

            # x[1] DMAs queue right behind wdyn on the DMA engines
            load_x(1)

            # ---------------- conv ------------------------------------------
            def wsl(b, cb, pos, ob):
                off = pos * O + ob * 128
                return wmix[b][cb][:, off : off + 128]

            TILES = [(ob, rg) for ob in range(2) for rg in range(8)]
            GROUPS = [TILES[0:5], TILES[5:10], TILES[10:15], TILES[15:16]]

            def corr_block(b, ob, dep_mm):
                corr = corrps.tile([128, 128], f32, tag="corr", name=f"corr{b}{ob}")
                first = True
                for side, dxv in ((0, 0), (1, 2)):
                    i = 0
                    for cb in range(2):
                        for dy in range(KS):
                            s = dy * W + (0 if side == 0 else ROW0)
                            rhs = xpad[b][cb][:, s : s + (H - 1) * W + 1 : W]
                            mm = nc.tensor.matmul(
                                corr[:, side * 64 : side * 64 + 64],
                                wsl(b, cb, dy * KS + dxv, ob),
                                rhs,
                                start=(i == 0),
                                stop=(i == 5),
                            )
                            if first and dep_mm is not None:
                                add_dep_helper(mm.ins, dep_mm.ins, sync=True,
                                               reason="PE order: corr")
                            first = False
                            i += 1
                return corr

            def conv_sample(b):
                corr = {}
                for group in GROUPS:
                    cps = {}
                    for ob, rg in group:
                        cps[(ob, rg)] = convps.tile(
                            [128, 512], f32, tag="conv", name=f"cps{b}{ob}{rg}"
                        )
                    last_mm = None

                    # 6 passes: (cb, dy) in mix-chunk production order; the
                    # last (cb1, dy2) pass is tile-major so tiles retire early.
                    for cb in range(2):
                        for dy in range(KS):
                            final = cb == 1 and dy == KS - 1
                            if final:
                                order = [
                                    (ob, rg, pos)
                                    for ob, rg in group
                                    for pos in range(dy * KS, dy * KS + KS)
                                ]
                            else:
                                order = [
                                    (ob, rg, pos)
                                    for pos in range(dy * KS, dy * KS + KS)
                                    for ob, rg in group
                                ]
                            for ob, rg, pos in order:
                                ddy, dx = divmod(pos, 3)
                                s = (rg * 8 + ddy) * W + dx
                                last_mm = nc.tensor.matmul(
                                    cps[(ob, rg)],
                                    wsl(b, cb, pos, ob),
                                    xpad[b][cb][:, s : s + 512],
                                    start=(cb == 0 and pos == 0),
                                    stop=(final and pos == NPOS - 1),
                                )
                            # border corrections once per ob, after cb1 dy0
                            # (all wmix ready by then, subs come later)
                            if cb == 1 and dy == 0:
                                for ob in sorted({ob for ob, _ in group}):
                                    if (b, ob) not in corr:
                                        corr[(b, ob)] = corr_block(b, ob, last_mm)

                    for ob, rg in group:
                        y0 = rg * 8
                        osb = osbp.tile([128, 512], f32, tag="osb", name=f"osb{b}{ob}{rg}")
                        nc.scalar.copy(osb, cps[(ob, rg)])
                        ov = osb.rearrange("m (y x) -> m y x", x=W)[:, :, 0 : W : W - 1]
                        cv = corr[(b, ob)].rearrange("m (s y) -> m y s", s=2)[:, y0 : y0 + 8, :]
                        nc.vector.tensor_sub(ov, ov, cv)
                        nc.sync.dma_start(
                            out_d[b, ob * 128 : (ob + 1) * 128, y0 : y0 + 8, :],
                            osb.rearrange("m (y x) -> m y x", x=W),
                        )
                    yield last_mm

            g0 = conv_sample(0)
            next(g0)  # G1
            g2_last = next(g0)  # G2

            # sample-1 attention: pools on ACT (DVE mixes), MLP matmuls pinned
            # behind b0's G2 on the PE stream so they never stall it.
            pooled1 = pool_sample(1, ("act", "act"))
            attn_bc1 = attn_mlp(1, pooled1, g2_last)
            mix_sample(1, attn_bc1)

            for _ in g0:  # G3, G4
                pass
            for _ in conv_sample(1):
                pass

    nc.compile()
    return nc


def get_nc():
    if "nc" not in _CACHE:
        _CACHE["nc"] = _build_nc()
    return _CACHE["nc"]


def prep_inputs(x, w_dyn, fc1_w, fc1_b, fc2_w, fc2_b):
    """Host-side layout prep + batch sharding -> per-core input maps."""
    import ml_dtypes

    bf16 = ml_dtypes.bfloat16
    w_dynT = np.ascontiguousarray(
        np.transpose(np.asarray(w_dyn, np.float32), (0, 3, 4, 2, 1)).reshape(K, NPOS, C, O)
    ).astype(bf16)
    fc1wT = np.ascontiguousarray(np.asarray(fc1_w, np.float32).T) / float(H * W)
    fc1b = np.ascontiguousarray(np.asarray(fc1_b, np.float32).reshape(1, MID))
    fc2aug = np.ascontiguousarray(
        np.vstack([np.asarray(fc2_w, np.float32).T, np.asarray(fc2_b, np.float32)[None, :]])
        * INV_DELTA
    )
    x = np.asarray(x, np.float32).astype(bf16)
    in_maps = []
    for core in range(NCORES):
        in_maps.append(
            {
                "x": np.ascontiguousarray(x[core * NB : (core + 1) * NB]),
                "wdynT": w_dynT,
                "fc1wT": fc1wT,
                "fc1b": fc1b,
                "fc2aug": fc2aug,
            }
        )
    return in_maps


def kernel(x, w_dyn, fc1_w, fc1_b, fc2_w, fc2_b):
    from concourse.bass_utils import run_bass_kernel_spmd

    nc = get_nc()
    in_maps = prep_inputs(x, w_dyn, fc1_w, fc1_b, fc2_w, fc2_b)
    res = run_bass_kernel_spmd(nc, in_maps, core_ids=list(range(NCORES)))
    return np.concatenate([r["out"] for r in res.results], axis=0)


# revision 12
# speedup vs baseline: 1.1961x; 1.1961x over previous
"""Trainium2 Bass kernel for nn_ConvDY2d (dynamic-weight 3x3 conv, CondConv-style).

Reference computation (B=16, C=O=256, H=W=64, K=4 mixing kernels):
  attn  = softmax(MLP(global_avg_pool(x)) / 30)            # [B, 4]
  w_mix = einsum('bk,koihw->boihw', attn, w_dyn)           # per-sample 3x3 conv kernel
  out[b] = conv2d(x[b], w_mix[b], padding=1)

Strategy: data-parallel over batch, 2 samples per NeuronCore across 8 cores.
Per core, the conv is an implicit GEMM: for each (out-channel block, 8-row
group) a [128, 512] PSUM tile accumulates 18 matmuls (2 c-blocks x 9 taps)
whose rhs are contiguous 512-element slices of a row-padded input image
([128c, 4226]).  Column wrap-around at row edges is fixed up afterwards by
subtracting border corrections computed with 12 strided-rhs matmuls per
output block.

Startup-latency-optimized schedule:
  - x[b0] loads first (row chunks); pooling partials chase the DMA chunks
    (DVE cb0 / ACT cb1), dep-chained in order so the tile scheduler cannot
    reorder them behind later DMAs that recycle the same semaphores.
  - wdyn is loaded split by (cblock, dy-row, k): the first mix chunk only
    needs 0.75MB of wdyn, so conv matmuls start ~19us instead of ~32us.
  - weight mixing uses tensor_scalar (4x DVE mode) + tensor_tensor (2x)
    trees in bf16: 7 ops/chunk instead of a 4-long scalar_tensor_tensor
    chain at 1x.
  - the conv runs in PSUM groups of 5 tiles with per-(cb,dy) passes so the
    matmul stream chases the mix chunks without stalling.
"""

import sys

if "/opt/trn_rl_repo" not in sys.path:
    sys.path.insert(0, "/opt/trn_rl_repo")

import numpy as np

B, C, H, W = 16, 256, 64, 64
O, K, KS = 256, 4, 3
MID = C // 4
INV_DELTA = 1.0 / 30.0
NCORES = 8
NB = B // NCORES            # samples per core
NPOS = KS * KS              # 9 taps
FPAD = 1 + 66 * W + 1       # padded image free size: 4226
ROW0 = 65                   # flat offset of input row 0 (= 1 + 1*64)

# x chunk row-splits per c-block: small final chunk so the last pooling
# partial is cheap and attention comes off the critical path.
XCHUNKS = [(0, 21), (21, 21), (42, 21), (63, 1)]

_CACHE = {}


def _build_nc():
    import concourse.bacc as bacc
    import concourse.tile as tile
    from concourse import mybir
    from concourse.tile_rust import add_dep_helper

    f32 = mybir.dt.float32
    bf16 = mybir.dt.bfloat16
    AX = mybir.AxisListType
    ALU = mybir.AluOpType
    ACTF = mybir.ActivationFunctionType

    nc = bacc.Bacc(target_bir_lowering=False, debug=False)

    x_d = nc.dram_tensor("x", [NB, C, H, W], bf16, kind="ExternalInput").ap()
    wd_d = nc.dram_tensor("wdynT", [K, NPOS, C, O], bf16, kind="ExternalInput").ap()
    fc1wT_d = nc.dram_tensor("fc1wT", [C, MID], f32, kind="ExternalInput").ap()
    fc1b_d = nc.dram_tensor("fc1b", [1, MID], f32, kind="ExternalInput").ap()
    fc2aug_d = nc.dram_tensor("fc2aug", [MID + 1, K], f32, kind="ExternalInput").ap()
    out_d = nc.dram_tensor("out", [NB, O, H, W], f32, kind="ExternalOutput").ap()

    with tile.TileContext(nc) as tc:
        with (
            tc.tile_pool(name="consts", bufs=1) as constp,
            tc.tile_pool(name="wdyn", bufs=1) as wdynp,
            tc.tile_pool(name="wmix", bufs=1) as wmixp,
            tc.tile_pool(name="xpad", bufs=1) as xpadp,
            tc.tile_pool(name="osb", bufs=6) as osbp,
            tc.tile_pool(name="convps", bufs=5, space="PSUM") as convps,
            tc.tile_pool(name="corrps", bufs=2, space="PSUM") as corrps,
            tc.tile_pool(name="smallps", bufs=1, space="PSUM") as smallps,
        ):
            ones_sb = constp.tile([1, 128], f32, tag="ones", name="ones_sb")
            nc.gpsimd.memset(ones_sb, 1.0)
            act_dummy = constp.tile([128, 21 * W], bf16, tag="actdum", name="act_dummy")

            # xpad tiles + pad memsets for both samples up front (gpsimd idle)
            xpad = [[None, None] for _ in range(NB)]
            for b in range(NB):
                for cb in range(2):
                    t = xpadp.tile([128, FPAD], bf16, tag=f"xpad{b}{cb}", name=f"xpad{b}{cb}")
                    nc.gpsimd.memset(t[:, 0:ROW0], 0.0)
                    nc.gpsimd.memset(t[:, ROW0 + H * W : FPAD], 0.0)
                    xpad[b][cb] = t

            def load_x(b):
                # interleave c-block chunks so both pooling engines get data
                for r0, nr in XCHUNKS:
                    for cb in range(2):
                        nc.sync.dma_start(
                            xpad[b][cb][:, ROW0 + r0 * W : ROW0 + (r0 + nr) * W],
                            x_d[
                                b, cb * 128 : (cb + 1) * 128, r0 : r0 + nr, :
                            ].rearrange("c h w -> c (h w)"),
                        )

            load_x(0)

            # MLP consts load behind x[b0] on the DMA queue (needed ~4us later
            # than pooling data, so x goes first)
            fc1wT_sb = constp.tile([128, 2 * MID], f32, tag="fc1w", name="fc1wT_sb")
            for cb in range(2):
                nc.sync.dma_start(
                    fc1wT_sb[:, cb * MID : (cb + 1) * MID],
                    fc1wT_d[cb * 128 : (cb + 1) * 128, :],
                )
            fc1b_sb = constp.tile([1, MID], f32, tag="fc1b", name="fc1b_sb")
            nc.sync.dma_start(fc1b_sb, fc1b_d)
            fc2aug_sb = constp.tile([MID + 1, K], f32, tag="fc2", name="fc2aug_sb")
            nc.sync.dma_start(fc2aug_sb, fc2aug_d)

            # ---------------- pooling + attention (sample 0) -----------------
            # Partials are dep-chained in chunk order: the scheduler would
            # otherwise reorder them and regenerate semaphore targets against
            # later DMAs that recycle the same semaphores (false deps).
            def pool_sample(b, engines):
                pooled = [None, None]
                nch = len(XCHUNKS)
                prev = [None, None]
                for h, (r0, nr) in enumerate(XCHUNKS):
                    for cb in range(2):
                        if h == 0:
                            pp = constp.tile(
                                [128, nch], f32, tag=f"pp{b}{cb}", name=f"pp{b}{cb}"
                            )
                            if cb == 0:
                                pp0 = pp
                            else:
                                pp1 = pp
                        pp = pp0 if cb == 0 else pp1
                        src = xpad[b][cb][:, ROW0 + r0 * W : ROW0 + (r0 + nr) * W]
                        if engines[cb] == "dve":
                            r = nc.vector.reduce_sum(pp[:, h : h + 1], src, AX.X)
                        else:
                            r = nc.scalar.activation(
                                act_dummy[:, 0 : nr * W], src, ACTF.Copy,
                                accum_out=pp[:, h : h + 1],
                            )
                        if prev[cb] is not None:
                            add_dep_helper(r.ins, prev[cb].ins, sync=False,
                                           reason="pool chunk order")
                        prev[cb] = r
                for cb in range(2):
                    pp = pp0 if cb == 0 else pp1
                    p = constp.tile([128, 1], f32, tag=f"pool{b}{cb}", name=f"pooled{b}{cb}")
                    r = nc.vector.reduce_sum(p, pp, AX.X)
                    if engines[cb] == "dve":
                        add_dep_helper(r.ins, prev[cb].ins, sync=False,
                                       reason="pool final order")
                    pooled[cb] = p
                return pooled

            def attn_mlp(b, pooled, first_dep):
                hid_ps = smallps.tile([MID, 1], f32, tag="small", name=f"hid_ps{b}")
                first_mm = None
                for cb in range(2):
                    mm = nc.tensor.matmul(
                        hid_ps,
                        fc1wT_sb[:, cb * MID : (cb + 1) * MID],
                        pooled[cb],
                        start=(cb == 0),
                        stop=False,
                    )
                    if first_mm is None:
                        first_mm = mm
                        if first_dep is not None:
                            add_dep_helper(mm.ins, first_dep.ins, sync=False,
                                           reason="PE order for MLP")
                nc.tensor.matmul(hid_ps, fc1b_sb, ones_sb[:, 0:1], start=False, stop=True)

                hid_sb = constp.tile([MID + 1, 1], f32, tag=f"hid{b}", name=f"hid_sb{b}")
                nc.gpsimd.memset(hid_sb[MID : MID + 1, :], 1.0)
                nc.scalar.activation(hid_sb[0:MID, :], hid_ps, ACTF.Relu)

                lg_ps = smallps.tile([1, K], f32, tag="small", name=f"lg_ps{b}")
                nc.tensor.matmul(lg_ps, hid_sb, fc2aug_sb, start=True, stop=True)

                ex = constp.tile([1, K], f32, tag=f"ex{b}", name=f"ex{b}")
                sm = constp.tile([1, 1], f32, tag=f"sm{b}", name=f"sm{b}")
                nc.scalar.activation(ex, lg_ps, ACTF.Exp, accum_out=sm)
                rc = constp.tile([1, 1], f32, tag=f"rc{b}", name=f"rc{b}")
                nc.vector.reciprocal(rc, sm)
                attn = constp.tile([1, K], f32, tag=f"at{b}", name=f"attn{b}")
                nc.vector.tensor_scalar_mul(attn, ex, rc)
                attn_bc = constp.tile([128, K], f32, tag=f"abc{b}", name=f"attn_bc{b}")
                nc.gpsimd.partition_broadcast(attn_bc, attn)
                return attn_bc

            pooled0 = pool_sample(0, ("dve", "act"))
            attn_bc0 = attn_mlp(0, pooled0, None)

            # ---------------- wdyn loads, split by (cb, dy-row, k) -----------
            # First 4 DMAs (cb0 dy0, all k) are all the first mix chunk needs.
            wdyn = [[None, None] for _ in range(K)]
            for cb in range(2):
                for k in range(K):
                    wdyn[k][cb] = wdynp.tile(
                        [128, NPOS * O], bf16, tag=f"wd{k}{cb}", name=f"wd{k}{cb}"
                    )
            for cb in range(2):
                for dy in range(KS):
                    for k in range(K):
                        nc.sync.dma_start(
                            wdyn[k][cb].rearrange("c (p o) -> c p o", o=O)[
                                :, dy * KS : (dy + 1) * KS, :
                            ],
                            wd_d[
                                k, dy * KS : (dy + 1) * KS, cb * 128 : (cb + 1) * 128, :
                            ].transpose([1, 0, 2]),
                        )

            # ---------------- weight mixing: bf16 ts/tt trees on DVE ---------
            # Per (cb, dy) chunk of [128, 768]:
            #   wm = a0*w0; s1 = a1*w1; wm += s1; s1 = a2*w2; s2 = a3*w3;
            #   s1 += s2; wm += s1          (4x tensor_scalar, 2x tensor_tensor)
            mix_s1 = constp.tile([128, KS * O], bf16, tag="mixs1", name="mix_s1")
            mix_s2 = constp.tile([128, KS * O], bf16, tag="mixs2", name="mix_s2")
            wmix = [[None, None] for _ in range(NB)]
            mix_last = [None]

            def mix_sample(b, attn_bc):
                for cb in range(2):
                    wm = wmixp.tile(
                        [128, NPOS * O], bf16, tag=f"wm{b}{cb}", name=f"wmix{b}{cb}"
                    )
                    for dy in range(KS):
                        lo, hi = dy * KS * O, (dy + 1) * KS * O
                        wmh = wm[:, lo:hi]
                        first = nc.vector.tensor_scalar_mul(
                            wmh, wdyn[0][cb][:, lo:hi], attn_bc[:, 0:1]
                        )
                        if mix_last[0] is not None:
                            add_dep_helper(first.ins, mix_last[0].ins, sync=False,
                                           reason="mix chunk order")
                        nc.vector.tensor_scalar_mul(
                            mix_s1, wdyn[1][cb][:, lo:hi], attn_bc[:, 1:2]
                        )
                        nc.vector.tensor_tensor(wmh, wmh, mix_s1, op=ALU.add)
                        nc.vector.tensor_scalar_mul(
                            mix_s1, wdyn[2][cb][:, lo:hi], attn_bc[:, 2:3]
                        )
                        nc.vector.tensor_scalar_mul(
                            mix_s2, wdyn[3][cb][:, lo:hi], attn_bc[:, 3:4]
                        )
                        nc.vector.tensor_tensor(mix_s1, mix_s1, mix_s2, op=ALU.add)
                        mix_last[0] = nc.vector.tensor_tensor(
                            wmh, wmh, mix_s1, op=ALU.add
                        )
                    wmix[b][cb] = wm

            mix_sample(0, attn_bc0)

            # x[1] DMAs queue right behind wdyn on the DMA engines
            load_x(1)

            # ---------------- conv ------------------------------------------
            def wsl(b, cb, pos, ob):
                off = pos * O + ob * 128
                return wmix[b][cb][:, off : off + 128]

            TILES = [(ob, rg) for ob in range(2) for rg in range(8)]
            GROUPS = [TILES[0:5], TILES[5:10], TILES[10:15], TILES[15:16]]

            def corr_block(b, ob, dep_mm):
                corr = corrps.tile([128, 128], f32, tag="corr", name=f"corr{b}{ob}")
                first = True
                for side, dxv in ((0, 0), (1, 2)):
                    i = 0
                    for cb in range(2):
                        for dy in range(KS):
                            s = dy * W + (0 if side == 0 else ROW0)
                            rhs = xpad[b][cb][:, s : s + (H - 1) * W + 1 : W]
                            mm = nc.tensor.matmul(
                                corr[:, side * 64 : side * 64 + 64],
                                wsl(b, cb, dy * KS + dxv, ob),
                                rhs,
                                start=(i == 0),
                                stop=(i == 5),
                            )
                            if first and dep_mm is not None:
                                add_dep_helper(mm.ins, dep_mm.ins, sync=False,
                                               reason="PE order: corr")
                            first = False
                            i += 1
                return corr

            def conv_sample(b):
                corr = {}
                for group in GROUPS:
                    cps = {}
                    for ob, rg in group:
                        cps[(ob, rg)] = convps.tile(
                            [128, 512], f32, tag="conv", name=f"cps{b}{ob}{rg}"
                        )
                    last_mm = None

                    # 6 passes: (cb, dy) in mix-chunk production order; the
                    # last (cb1, dy2) pass is tile-major so tiles retire early.
                    for cb in range(2):
                        for dy in range(KS):
                            final = cb == 1 and dy == KS - 1
                            if final:
                                order = [
                                    (ob, rg, pos)
                                    for ob, rg in group
                                    for pos in range(dy * KS, dy * KS + KS)
                                ]
                            else:
                                order = [
                                    (ob, rg, pos)
                                    for pos in range(dy * KS, dy * KS + KS)
                                    for ob, rg in group
                                ]
                            for ob, rg, pos in order:
                                ddy, dx = divmod(pos, 3)
                                s = (rg * 8 + ddy) * W + dx
                                last_mm = nc.tensor.matmul(
                                    cps[(ob, rg)],
                                    wsl(b, cb, pos, ob),
                                    xpad[b][cb][:, s : s + 512],
                                    start=(cb == 0 and pos == 0),
                                    stop=(final and pos == NPOS - 1),
                                )
                            # border corrections once per ob, after cb1 dy0
                            # (all wmix ready by then, subs come later)
                            if cb == 1 and dy == 0:
                                for ob in sorted({ob for ob, _ in group}):
                                    if (b, ob) not in corr:
                                        corr[(b, ob)] = corr_block(b, ob, last_mm)

                    for ob, rg in group:
                        y0 = rg * 8
                        osb = osbp.tile([128, 512], f32, tag="osb", name=f"osb{b}{ob}{rg}")
                        nc.scalar.copy(osb, cps[(ob, rg)])
                        ov = osb.rearrange("m (y x) -> m y x", x=W)[:, :, 0 : W : W - 1]
                        cv = corr[(b, ob)].rearrange("m (s y) -> m y s", s=2)[:, y0 : y0 + 8, :]
                        nc.vector.tensor_sub(ov, ov, cv)
                        nc.sync.dma_start(
                            out_d[b, ob * 128 : (ob + 1) * 128, y0 : y0 + 8, :],
                            osb.rearrange("m (y x) -> m y x", x=W),
                        )
                    yield last_mm

            g0 = conv_sample(0)
            next(g0)  # G1
            g2_last = next(g0)  # G2

            # sample-1 attention: pools on ACT (DVE mixes), MLP matmuls pinned
            # behind b0's G2 on the PE stream so they never stall it.
            pooled1 = pool_sample(1, ("act", "act"))
            attn_bc1 = attn_mlp(1, pooled1, g2_last)
            mix_sample(1, attn_bc1)

            for _ in g0:  # G3, G4
                pass
            for _ in conv_sample(1):
                pass

    nc.compile()
    return nc


def get_nc():
    if "nc" not in _CACHE:
        _CACHE["nc"] = _build_nc()
    return _CACHE["nc"]


def prep_inputs(x, w_dyn, fc1_w, fc1_b, fc2_w, fc2_b):
    """Host-side layout prep + batch sharding -> per-core input maps."""
    import ml_dtypes

    bf16 = ml_dtypes.bfloat16
    w_dynT = np.ascontiguousarray(
        np.transpose(np.asarray(w_dyn, np.float32), (0, 3, 4, 2, 1)).reshape(K, NPOS, C, O)
    ).astype(bf16)
    fc1wT = np.ascontiguousarray(np.asarray(fc1_w, np.float32).T) / float(H * W)
    fc1b = np.ascontiguousarray(np.asarray(fc1_b, np.float32).reshape(1, MID))
    fc2aug = np.ascontiguousarray(
        np.vstack([np.asarray(fc2_w, np.float32).T, np.asarray(fc2_b, np.float32)[None, :]])
        * INV_DELTA
    )
    x = np.asarray(x, np.float32).astype(bf16)
    in_maps = []
    for core in range(NCORES):
        in_maps.append(
            {
                "x": np.ascontiguousarray(x[core * NB : (core + 1) * NB]),
                "wdynT": w_dynT,
                "fc1wT": fc1wT,
                "fc1b": fc1b,
                "fc2aug": fc2aug,
            }
        )
    return in_maps


def kernel(x, w_dyn, fc1_w, fc1_b, fc2_w, fc2_b):
    from concourse.bass_utils import run_bass_kernel_spmd

    nc = get_nc()
    in_maps = prep_inputs(x, w_dyn, fc1_w, fc1_b, fc2_w, fc2_b)
    res = run_bass_kernel_spmd(nc, in_maps, core_ids=list(range(NCORES)))
    return np.concatenate([r["out"] for r in res.results], axis=0)
